# revision 10
# baseline (speedup 1.0000x reference)
# Trainium2 Bass kernel for nn_MCorrLCorr (Mellin-correlation along x,
# linear correlation along y).
#
#   out[b,o,hx,hy] = bias[o]
#     + sum_{c,fx,fy} input[b, c, (hx+1)*(fx+1)-1, 2*hy + fy - 2] * weight[o,c,fx,fy]
#   (terms with 2*hy+fy-2 < 0 dropped; only hy=0, fy<2)
#
# The x-gather, fp32->bf16 cast and even/odd-gy parity split are pure data
# movement, so they are done on the HOST (numpy) and the device receives the
# input already in matmul layout:
#   xg[b, ch, q, (fx,c)=128, l=16, col=194] bf16 with
#     col 1+t = input[b, c, (ch*16+l+1)*(fx+1)-1, 2t+q], cols 0/193 = zero
#     (the zero edge columns absorb the dropped out-of-range y terms).
# This more than halves HBM traffic vs an on-chip f32 gather and frees
# ACT/DVE from cast work; the output returns as bf16 and is upcast on host.
#
# Per core (2 batches, data-parallel over 8 cores), per 16-hx chunk:
#   1. input tiles stream over the sync DMA ring in consumption order (each
#      dma_start fans out over all 16 DMA engines, so queueing them on one
#      ring makes tile k complete before tile k+1). The first tile is split
#      across sync+scalar so the first matmul starts ~1.3us earlier.
#   2. matmul: same-parity fy pairs (fy, fy+2) share one moving stream
#      shifted by one hy. With stationary [W_fy | W_fy+2] (K=128 x M=128,
#      full PE array) a single bf16 matmul over xq[:, l0:l0+2, off:off+192]
#      (N=384) computes both fy: PSUM rows 0:64 hold fy_lo sums at hy=n,
#      rows 64:128 hold fy_hi sums at hy=n-1. Loop order is BANK-major (all
#      4 pairs accumulate into one bank back-to-back) so each bank's
#      combine can start 4 matmuls after the bank begins -- the combine
#      pipeline drains alongside the matmul stream instead of after it.
#      LDWEIGHTS (96ns, background weight plane) stays hidden under the
#      163ns matmuls either way.
#   3. combine: ACT adds bias while copying rows 0:64 (PSUM->SBUF, casting
#      bf16), DVE adds the hy-shifted rows 64:128. One output DMA per two
#      banks (4 hx rows) on the scalar HWDGE ring (SWDGE via gpsimd is ~4x
#      slower and would add ~2us of tail drain).

import ml_dtypes
import numpy as np

import concourse.bass as bass
import concourse.mybir as mybir
import concourse.tile as tile
from concourse import bacc
from concourse.bass_utils import run_bass_kernel_spmd

B, C, NGX, NGY = 16, 32, 128, 384
O, NFX, NFY = 64, 4, 8
NHX, NHY = 32, 190
NCORES = 8
BPC = B // NCORES  # batches per core
F32 = mybir.dt.float32
BF16 = mybir.dt.bfloat16

P = NFX * C  # partition dim of the gathered input (128)
HX_TILE = 2  # output hx rows per PSUM bank slot
NMM = NHY + 2  # moving columns per matmul per hx row (192)
NPAR = NHY + 4  # parity-tile columns: [zero, 192 gy values, zero]
PAIR_LO = (0, 4, 1, 5)  # fy pairs (lo, lo+2); even-parity pairs first
NSLOT = len(PAIR_LO)  # 4 fy pairs
NGRP = 8  # PSUM bank slots per chunk
HCH = NGRP * HX_TILE  # hx rows per chunk (16)
NCHUNK = NHX // HCH  # chunks per batch (2)
ODMA_BANKS = 2  # banks (4 hx rows) per output DMA


def build_nc():
    nc = bacc.Bacc("TRN2", target_bir_lowering=False)
    xg = nc.dram_tensor(
        "xg", [BPC, NCHUNK, 2, P, HCH, NPAR], BF16, kind="ExternalInput"
    )
    wre = nc.dram_tensor("weight", [P, NSLOT, 128], BF16, kind="ExternalInput")
    bia = nc.dram_tensor("bias", [O, 1], F32, kind="ExternalInput")
    out = nc.dram_tensor("out", [BPC, O, NHX, NHY], BF16, kind="ExternalOutput")
    xg_ap, wre_ap, bia_ap, out_ap = xg.ap(), wre.ap(), bia.ap(), out.ap()

    with tile.TileContext(nc) as tc:
        with (
            tc.tile_pool(name="consts", bufs=1) as consts,
            tc.tile_pool(name="xin", bufs=1) as xpool,
            tc.tile_pool(name="obc", bufs=3) as opool,
            tc.tile_pool(name="ps", bufs=8, space="PSUM") as pspool,
        ):
            w_sb = consts.tile([P, NSLOT, 128], BF16)
            nc.scalar.dma_start(out=w_sb, in_=wre_ap)
            bias_sb = consts.tile([O, 1], F32)
            nc.scalar.dma_start(out=bias_sb, in_=bia_ap)

            # input tiles in consumption order; tile 0 split across rings
            xts = {}
            first = True
            for b in range(BPC):
                for ch in range(NCHUNK):
                    for q in range(2):
                        xt = xpool.tile(
                            [P, HCH, NPAR],
                            BF16,
                            tag=f"x_{b}_{ch}_{q}",
                            name=f"x_{b}_{ch}_{q}",
                        )
                        if first:
                            nc.sync.dma_start(
                                out=xt[0 : P // 2], in_=xg_ap[b, ch, q, 0 : P // 2]
                            )
                            nc.scalar.dma_start(
                                out=xt[P // 2 : P], in_=xg_ap[b, ch, q, P // 2 : P]
                            )
                            first = False
                        else:
                            nc.sync.dma_start(out=xt, in_=xg_ap[b, ch, q])
                        xts[(b, ch, q)] = xt

            for b in range(BPC):
                for ch in range(NCHUNK):
                    hxb = ch * HCH
                    pss = [
                        pspool.tile(
                            [128, HX_TILE, NMM], F32, tag="ps", name=f"ps_{b}_{ch}_{g}"
                        )
                        for g in range(NGRP)
                    ]
                    obc = opool.tile([O, HCH, NHY], BF16, tag="obc", name=f"obc_{b}_{ch}")
                    for g in range(NGRP):
                        l0 = g * HX_TILE
                        ps = pss[g]
                        for pr in range(NSLOT):
                            fy_lo = PAIR_LO[pr]
                            q, off = fy_lo & 1, (fy_lo - (fy_lo & 1)) // 2
                            nc.tensor.matmul(
                                ps,
                                w_sb[:, pr, :],
                                xts[(b, ch, q)][:, l0 : l0 + HX_TILE, off : off + NMM],
                                start=(pr == 0),
                                stop=(pr == NSLOT - 1),
                            )
                        ob = obc[:, l0 : l0 + HX_TILE, :]
                        # rows 0:64: fy_lo sums at hy=n; add bias while copying
                        # (DVE cannot read two PSUM operands in one op)
                        nc.scalar.add(ob, ps[0:O, :, 0:NHY], bias_sb)
                        # rows 64:128: fy_hi sums at hy=n-1 -> shift left by one
                        nc.vector.tensor_add(ob, ob, ps[O:128, :, 1 : NHY + 1])
                        if g % ODMA_BANKS == ODMA_BANKS - 1:
                            r0 = (g - ODMA_BANKS + 1) * HX_TILE
                            r1 = (g + 1) * HX_TILE
                            nc.scalar.dma_start(
                                out=out_ap[b, :, hxb + r0 : hxb + r1, :],
                                in_=obc[:, r0:r1, :],
                            )
    nc.compile()
    return nc


def _prep_maps(inputs):
    inp = np.asarray(inputs["input"], dtype=np.float32)
    w = np.asarray(inputs["weight"], dtype=np.float32)

    # x-gather: rows[fx, hx] = (hx+1)*(fx+1)-1
    fx = np.arange(NFX)[:, None]
    hx = np.arange(NHX)[None, :]
    rows = (hx + 1) * (fx + 1) - 1  # [NFX, NHX]
    g = inp[:, :, rows, :]  # [B, C, NFX, NHX, NGY]
    g = g.transpose(0, 2, 1, 3, 4).reshape(B, P, NHX, NGY)

    X = np.zeros((B, NCHUNK, 2, P, HCH, NPAR), dtype=ml_dtypes.bfloat16)
    for ch in range(NCHUNK):
        sl = g[:, :, ch * HCH : (ch + 1) * HCH, :]
        X[:, ch, 0, :, :, 1 : NPAR - 1] = sl[..., 0::2]
        X[:, ch, 1, :, :, 1 : NPAR - 1] = sl[..., 1::2]

    # wt[fx*C + c, fy, o] = weight[o, c, fx, fy]
    wt = w.transpose(2, 1, 3, 0).reshape(P, NFY, O)
    w2 = np.zeros((P, NSLOT, 128), np.float32)
    for pr, fy_lo in enumerate(PAIR_LO):
        w2[:, pr, 0:O] = wt[:, fy_lo]
        w2[:, pr, O:128] = wt[:, fy_lo + 2]
    w2 = np.ascontiguousarray(w2.astype(ml_dtypes.bfloat16))
    bre = np.ascontiguousarray(
        np.asarray(inputs["bias"], dtype=np.float32).reshape(O, 1)
    )
    return [
        {
            "xg": np.ascontiguousarray(X[k * BPC : (k + 1) * BPC]),
            "weight": w2,
            "bias": bre,
        }
        for k in range(NCORES)
    ]


def kernel(**inputs) -> np.ndarray:
    nc = build_nc()
    in_maps = _prep_maps(inputs)
    res = run_bass_kernel_spmd(nc, in_maps, core_ids=list(range(NCORES)))
    out = np.concatenate([r["out"] for r in res.results], axis=0)
    return out.astype(np.float32)


# revision 14
# speedup vs baseline: 1.0758x; 1.0758x over previous
# Trainium2 Bass kernel for nn_MCorrLCorr (Mellin-correlation along x,
# linear correlation along y).
#
#   out[b,o,hx,hy] = bias[o]
#     + sum_{c,fx,fy} input[b, c, (hx+1)*(fx+1)-1, 2*hy + fy - 2] * weight[o,c,fx,fy]
#   (terms with 2*hy+fy-2 < 0 dropped; only hy=0, fy<2)
#
# The x-gather, fp32->bf16 cast and even/odd-gy parity split are pure data
# movement, so they are done on the HOST (numpy) and the device receives the
# input already in matmul layout:
#   xg[b, ch, q, (fx,c)=128, l=16, col=194] bf16 with
#     col 1+t = input[b, c, (ch*16+l+1)*(fx+1)-1, 2t+q], cols 0/193 = zero
#     (the zero edge columns absorb the dropped out-of-range y terms).
# This more than halves HBM traffic vs an on-chip f32 gather and frees
# ACT/DVE from cast work; the output returns as bf16 and is upcast on host.
#
# Per core (2 batches, data-parallel over 8 cores), per 16-hx chunk:
#   1. input tiles stream over the sync DMA ring in consumption order (each
#      dma_start fans out over all 16 DMA engines, so queueing them on one
#      ring makes tile k complete before tile k+1). The first tile is split
#      across sync+scalar so the first matmul starts ~1.3us earlier.
#   2. matmul: same-parity fy pairs (fy, fy+2) share one moving stream
#      shifted by one hy. With stationary [W_fy | W_fy+2] (K=128 x M=128,
#      full PE array) a single bf16 matmul over xq[:, l0:l0+2, off:off+192]
#      (N=384) computes both fy: PSUM rows 0:64 hold fy_lo sums at hy=n,
#      rows 64:128 hold fy_hi sums at hy=n-1. Loop order: pr-major over
#      GROUPS of 4 banks (back-to-back matmuls must NOT accumulate into the
#      same PSUM bank -- that serializes each LDWEIGHTS with the previous
#      matmul and halves the PE rate; with a 4-bank sweep per stationary
#      the 96ns LDWEIGHTS hides under the 163ns matmuls). Group A's combine
#      drains while group B's matmuls run, so only the last group's combine
#      (~2us) trails the final matmul.
#   3. combine: ACT adds bias while copying rows 0:64 (PSUM->SBUF, casting
#      bf16), DVE adds the hy-shifted rows 64:128. One output DMA per group
#      (8 hx rows) on the sync HWDGE ring behind the inputs (SWDGE via
#      gpsimd is ~4x slower and would add ~2us of tail drain).

import ml_dtypes
import numpy as np

import concourse.bass as bass
import concourse.mybir as mybir
import concourse.tile as tile
from concourse import bacc
from concourse.bass_utils import run_bass_kernel_spmd

B, C, NGX, NGY = 16, 32, 128, 384
O, NFX, NFY = 64, 4, 8
NHX, NHY = 32, 190
NCORES = 8
BPC = B // NCORES  # batches per core
F32 = mybir.dt.float32
BF16 = mybir.dt.bfloat16

P = NFX * C  # partition dim of the gathered input (128)
HX_TILE = 2  # output hx rows per PSUM bank slot
NMM = NHY + 2  # moving columns per matmul per hx row (192)
NPAR = NHY + 4  # parity-tile columns: [zero, 192 gy values, zero]
PAIR_LO = (0, 4, 1, 5)  # fy pairs (lo, lo+2); even-parity pairs first
NSLOT = len(PAIR_LO)  # 4 fy pairs
NGRP = 8  # PSUM bank slots per chunk
GRP_SWEEP = 4  # banks swept per stationary load (pr-major within a group)
HCH = NGRP * HX_TILE  # hx rows per chunk (16)
NCHUNK = NHX // HCH  # chunks per batch (2)


def build_nc():
    nc = bacc.Bacc("TRN2", target_bir_lowering=False)
    xg = nc.dram_tensor(
        "xg", [BPC, NCHUNK, 2, P, HCH, NPAR], BF16, kind="ExternalInput"
    )
    wre = nc.dram_tensor("weight", [P, NSLOT, 128], BF16, kind="ExternalInput")
    bia = nc.dram_tensor("bias", [O, 1], F32, kind="ExternalInput")
    out = nc.dram_tensor("out", [BPC, O, NHX, NHY], BF16, kind="ExternalOutput")
    xg_ap, wre_ap, bia_ap, out_ap = xg.ap(), wre.ap(), bia.ap(), out.ap()

    with tile.TileContext(nc) as tc:
        with (
            tc.tile_pool(name="consts", bufs=1) as consts,
            tc.tile_pool(name="xin", bufs=1) as xpool,
            tc.tile_pool(name="obc", bufs=4) as opool,
            tc.tile_pool(name="ps", bufs=8, space="PSUM") as pspool,
        ):
            w_sb = consts.tile([P, NSLOT, 128], BF16)
            nc.scalar.dma_start(out=w_sb, in_=wre_ap)
            bias_sb = consts.tile([O, 1], F32)
            nc.scalar.dma_start(out=bias_sb, in_=bia_ap)

            # input tiles on the sync ring, in consumption order
            xts = {}
            for b in range(BPC):
                for ch in range(NCHUNK):
                    for q in range(2):
                        xt = xpool.tile(
                            [P, HCH, NPAR],
                            BF16,
                            tag=f"x_{b}_{ch}_{q}",
                            name=f"x_{b}_{ch}_{q}",
                        )
                        nc.sync.dma_start(out=xt, in_=xg_ap[b, ch, q])
                        xts[(b, ch, q)] = xt

            for b in range(BPC):
                for ch in range(NCHUNK):
                    hxb = ch * HCH
                    pss = [
                        pspool.tile(
                            [128, HX_TILE, NMM], F32, tag="ps", name=f"ps_{b}_{ch}_{g}"
                        )
                        for g in range(NGRP)
                    ]
                    obc = opool.tile([O, HCH, NHY], BF16, tag="obc", name=f"obc_{b}_{ch}")
                    for g0 in range(0, NGRP, GRP_SWEEP):
                        for pr in range(NSLOT):
                            fy_lo = PAIR_LO[pr]
                            q, off = fy_lo & 1, (fy_lo - (fy_lo & 1)) // 2
                            xt = xts[(b, ch, q)]
                            for g in range(g0, g0 + GRP_SWEEP):
                                l0 = g * HX_TILE
                                nc.tensor.matmul(
                                    pss[g],
                                    w_sb[:, pr, :],
                                    xt[:, l0 : l0 + HX_TILE, off : off + NMM],
                                    start=(pr == 0),
                                    stop=(pr == NSLOT - 1),
                                )
                        for g in range(g0, g0 + GRP_SWEEP):
                            l0 = g * HX_TILE
                            ps = pss[g]
                            ob = obc[:, l0 : l0 + HX_TILE, :]
                            # rows 0:64: fy_lo sums at hy=n; add bias while
                            # copying (DVE cannot read two PSUM operands)
                            nc.scalar.add(ob, ps[0:O, :, 0:NHY], bias_sb)
                            # rows 64:128: fy_hi at hy=n-1 -> shift left by one
                            nc.vector.tensor_add(ob, ob, ps[O:128, :, 1 : NHY + 1])
                        r0 = g0 * HX_TILE
                        r1 = (g0 + GRP_SWEEP) * HX_TILE
                        nc.sync.dma_start(
                            out=out_ap[b, :, hxb + r0 : hxb + r1, :],
                            in_=obc[:, r0:r1, :],
                        )
    nc.compile()
    return nc


def _prep_maps(inputs):
    inp = np.asarray(inputs["input"], dtype=np.float32)
    w = np.asarray(inputs["weight"], dtype=np.float32)

    # x-gather: rows[fx, hx] = (hx+1)*(fx+1)-1
    fx = np.arange(NFX)[:, None]
    hx = np.arange(NHX)[None, :]
    rows = (hx + 1) * (fx + 1) - 1  # [NFX, NHX]
    g = inp[:, :, rows, :]  # [B, C, NFX, NHX, NGY]
    g = g.transpose(0, 2, 1, 3, 4).reshape(B, P, NHX, NGY)

    X = np.zeros((B, NCHUNK, 2, P, HCH, NPAR), dtype=ml_dtypes.bfloat16)
    for ch in range(NCHUNK):
        sl = g[:, :, ch * HCH : (ch + 1) * HCH, :]
        X[:, ch, 0, :, :, 1 : NPAR - 1] = sl[..., 0::2]
        X[:, ch, 1, :, :, 1 : NPAR - 1] = sl[..., 1::2]

    # wt[fx*C + c, fy, o] = weight[o, c, fx, fy]
    wt = w.transpose(2, 1, 3, 0).reshape(P, NFY, O)
    w2 = np.zeros((P, NSLOT, 128), np.float32)
    for pr, fy_lo in enumerate(PAIR_LO):
        w2[:, pr, 0:O] = wt[:, fy_lo]
        w2[:, pr, O:128] = wt[:, fy_lo + 2]
    w2 = np.ascontiguousarray(w2.astype(ml_dtypes.bfloat16))
    bre = np.ascontiguousarray(
        np.asarray(inputs["bias"], dtype=np.float32).reshape(O, 1)
    )
    return [
        {
            "xg": np.ascontiguousarray(X[k * BPC : (k + 1) * BPC]),
            "weight": w2,
            "bias": bre,
        }
        for k in range(NCORES)
    ]


def kernel(**inputs) -> np.ndarray:
    nc = build_nc()
    in_maps = _prep_maps(inputs)
    res = run_bass_kernel_spmd(nc, in_maps, core_ids=list(range(NCORES)))
    out = np.concatenate([r["out"] for r in res.results], axis=0)
    return out.astype(np.float32)


# revision 16
# speedup vs baseline: 1.1065x; 1.0286x over previous
# Trainium2 Bass kernel for nn_MCorrLCorr (Mellin-correlation along x,
# linear correlation along y).
#
#   out[b,o,hx,hy] = bias[o]
#     + sum_{c,fx,fy} input[b, c, (hx+1)*(fx+1)-1, 2*hy + fy - 2] * weight[o,c,fx,fy]
#   (terms with 2*hy+fy-2 < 0 dropped; only hy=0, fy<2)
#
# The x-gather, fp32->bf16 cast and even/odd-gy parity split are pure data
# movement, so they are done on the HOST (numpy) and the device receives the
# input already in matmul layout:
#   xg[b, ch, q, (fx,c)=128, l=16, col=194] bf16 with
#     col 1+t = input[b, c, (ch*16+l+1)*(fx+1)-1, 2t+q], cols 0/193 = zero
#     (the zero edge columns absorb the dropped out-of-range y terms).
# This more than halves HBM traffic vs an on-chip f32 gather and frees
# ACT/DVE from cast work; the output returns as bf16 and is upcast on host.
#
# Per core (2 batches, data-parallel over 8 cores), per 16-hx chunk:
#   1. input tiles stream over the sync DMA ring in consumption order (each
#      dma_start fans out over all 16 DMA engines, so queueing them on one
#      ring makes tile k complete before tile k+1). The first tile is split
#      across sync+scalar so the first matmul starts ~1.3us earlier.
#   2. matmul: same-parity fy pairs (fy, fy+2) share one moving stream
#      shifted by one hy. With stationary [W_fy | W_fy+2] (K=128 x M=128,
#      full PE array) a single bf16 matmul over xq[:, l0:l0+2, off:off+192]
#      (N=384) computes both fy: PSUM rows 0:64 hold fy_lo sums at hy=n,
#      rows 64:128 hold fy_hi sums at hy=n-1. Loop order: pr-major over
#      GROUPS of 4 banks (back-to-back matmuls must NOT accumulate into the
#      same PSUM bank -- that serializes each LDWEIGHTS with the previous
#      matmul and halves the PE rate; with a 4-bank sweep per stationary
#      the 96ns LDWEIGHTS hides under the 163ns matmuls). Group A's combine
#      drains while group B's matmuls run, so only the last group's combine
#      (~2us) trails the final matmul.
#   3. combine: ACT adds bias while copying rows 0:64 (PSUM->SBUF, casting
#      bf16), DVE adds the hy-shifted rows 64:128. One output DMA per group
#      (8 hx rows) on the sync HWDGE ring behind the inputs (SWDGE via
#      gpsimd is ~4x slower and would add ~2us of tail drain).

import ml_dtypes
import numpy as np

import concourse.bass as bass
import concourse.mybir as mybir
import concourse.tile as tile
from concourse import bacc
from concourse.bass_utils import run_bass_kernel_spmd

B, C, NGX, NGY = 16, 32, 128, 384
O, NFX, NFY = 64, 4, 8
NHX, NHY = 32, 190
NCORES = 8
BPC = B // NCORES  # batches per core
F32 = mybir.dt.float32
BF16 = mybir.dt.bfloat16

P = NFX * C  # partition dim of the gathered input (128)
HX_TILE = 2  # output hx rows per PSUM bank slot
NMM = NHY + 2  # moving columns per matmul per hx row (192)
NPAR = NHY + 4  # parity-tile columns: [zero, 192 gy values, zero]
PAIR_LO = (0, 4, 1, 5)  # fy pairs (lo, lo+2); even-parity pairs first
NSLOT = len(PAIR_LO)  # 4 fy pairs
NGRP = 8  # PSUM bank slots per chunk
GRP_SWEEP = 4  # banks swept per stationary load (pr-major within a group)
HCH = NGRP * HX_TILE  # hx rows per chunk (16)
NCHUNK = NHX // HCH  # chunks per batch (2)


def build_nc():
    nc = bacc.Bacc("TRN2", target_bir_lowering=False)
    xg = nc.dram_tensor(
        "xg", [BPC, NCHUNK, 2, P, HCH, NPAR], BF16, kind="ExternalInput"
    )
    wre = nc.dram_tensor("weight", [P, NSLOT, 128], BF16, kind="ExternalInput")
    bia = nc.dram_tensor("bias", [O, 1], F32, kind="ExternalInput")
    out = nc.dram_tensor("out", [BPC, O, NHX, NHY], BF16, kind="ExternalOutput")
    xg_ap, wre_ap, bia_ap, out_ap = xg.ap(), wre.ap(), bia.ap(), out.ap()

    with tile.TileContext(nc) as tc:
        with (
            tc.tile_pool(name="consts", bufs=1) as consts,
            tc.tile_pool(name="xin", bufs=1) as xpool,
            tc.tile_pool(name="obc", bufs=4) as opool,
            tc.tile_pool(name="ps", bufs=8, space="PSUM") as pspool,
        ):
            # everything the first matmul needs goes FIRST on the sync ring
            # (on the scalar ring the weight DMA gets starved behind the
            # sync queue's input stream and delays the first matmul ~2.5us)
            w_sb = consts.tile([P, NSLOT, 128], BF16)
            nc.sync.dma_start(out=w_sb, in_=wre_ap)
            bias_sb = consts.tile([O, 1], F32)
            nc.sync.dma_start(out=bias_sb, in_=bia_ap)

            # input tiles on the sync ring, in consumption order; the first
            # chunk's tiles are split into half-chunk DMAs so group A of
            # chunk 0 (which only reads l 0:8) can start ~1us earlier
            xts = {}
            for b in range(BPC):
                for ch in range(NCHUNK):
                    for q in range(2):
                        xts[(b, ch, q)] = xpool.tile(
                            [P, HCH, NPAR],
                            BF16,
                            tag=f"x_{b}_{ch}_{q}",
                            name=f"x_{b}_{ch}_{q}",
                        )
            hh = HCH // 2
            for half in range(2):
                for q in range(2):
                    xt = xts[(0, 0, q)]
                    nc.sync.dma_start(
                        out=xt[:, half * hh : (half + 1) * hh, :],
                        in_=xg_ap[0, 0, q, :, half * hh : (half + 1) * hh, :],
                    )
            for b in range(BPC):
                for ch in range(NCHUNK):
                    if (b, ch) == (0, 0):
                        continue
                    for q in range(2):
                        nc.sync.dma_start(out=xts[(b, ch, q)], in_=xg_ap[b, ch, q])

            for b in range(BPC):
                for ch in range(NCHUNK):
                    hxb = ch * HCH
                    pss = [
                        pspool.tile(
                            [128, HX_TILE, NMM], F32, tag="ps", name=f"ps_{b}_{ch}_{g}"
                        )
                        for g in range(NGRP)
                    ]
                    obc = opool.tile([O, HCH, NHY], BF16, tag="obc", name=f"obc_{b}_{ch}")
                    for g0 in range(0, NGRP, GRP_SWEEP):
                        for pr in range(NSLOT):
                            fy_lo = PAIR_LO[pr]
                            q, off = fy_lo & 1, (fy_lo - (fy_lo & 1)) // 2
                            xt = xts[(b, ch, q)]
                            for g in range(g0, g0 + GRP_SWEEP):
                                l0 = g * HX_TILE
                                nc.tensor.matmul(
                                    pss[g],
                                    w_sb[:, pr, :],
                                    xt[:, l0 : l0 + HX_TILE, off : off + NMM],
                                    start=(pr == 0),
                                    stop=(pr == NSLOT - 1),
                                )
                        for g in range(g0, g0 + GRP_SWEEP):
                            l0 = g * HX_TILE
                            ps = pss[g]
                            ob = obc[:, l0 : l0 + HX_TILE, :]
                            # rows 0:64: fy_lo sums at hy=n; add bias while
                            # copying (DVE cannot read two PSUM operands)
                            nc.scalar.add(ob, ps[0:O, :, 0:NHY], bias_sb)
                            # rows 64:128: fy_hi at hy=n-1 -> shift left by one
                            nc.vector.tensor_add(ob, ob, ps[O:128, :, 1 : NHY + 1])
                        for g in range(g0, g0 + GRP_SWEEP, 2):
                            r0 = g * HX_TILE
                            r1 = (g + 2) * HX_TILE
                            nc.sync.dma_start(
                                out=out_ap[b, :, hxb + r0 : hxb + r1, :],
                                in_=obc[:, r0:r1, :],
                            )
    nc.compile()
    return nc


def _prep_maps(inputs):
    inp = np.asarray(inputs["input"], dtype=np.float32)
    w = np.asarray(inputs["weight"], dtype=np.float32)

    # x-gather: rows[fx, hx] = (hx+1)*(fx+1)-1
    fx = np.arange(NFX)[:, None]
    hx = np.arange(NHX)[None, :]
    rows = (hx + 1) * (fx + 1) - 1  # [NFX, NHX]
    g = inp[:, :, rows, :]  # [B, C, NFX, NHX, NGY]
    g = g.transpose(0, 2, 1, 3, 4).reshape(B, P, NHX, NGY)

    X = np.zeros((B, NCHUNK, 2, P, HCH, NPAR), dtype=ml_dtypes.bfloat16)
    for ch in range(NCHUNK):
        sl = g[:, :, ch * HCH : (ch + 1) * HCH, :]
        X[:, ch, 0, :, :, 1 : NPAR - 1] = sl[..., 0::2]
        X[:, ch, 1, :, :, 1 : NPAR - 1] = sl[..., 1::2]

    # wt[fx*C + c, fy, o] = weight[o, c, fx, fy]
    wt = w.transpose(2, 1, 3, 0).reshape(P, NFY, O)
    w2 = np.zeros((P, NSLOT, 128), np.float32)
    for pr, fy_lo in enumerate(PAIR_LO):
        w2[:, pr, 0:O] = wt[:, fy_lo]
        w2[:, pr, O:128] = wt[:, fy_lo + 2]
    w2 = np.ascontiguousarray(w2.astype(ml_dtypes.bfloat16))
    bre = np.ascontiguousarray(
        np.asarray(inputs["bias"], dtype=np.float32).reshape(O, 1)
    )
    return [
        {
            "xg": np.ascontiguousarray(X[k * BPC : (k + 1) * BPC]),
            "weight": w2,
            "bias": bre,
        }
        for k in range(NCORES)
    ]


def kernel(**inputs) -> np.ndarray:
    nc = build_nc()
    in_maps = _prep_maps(inputs)
    res = run_bass_kernel_spmd(nc, in_maps, core_ids=list(range(NCORES)))
    out = np.concatenate([r["out"] for r in res.results], axis=0)
    return out.astype(np.float32)
